# revision 41
# baseline (speedup 1.0000x reference)
"""Trainium2 Bass kernel for the consistency-loss problem.

loss = -mean_b( table[argmax_c pred1[b,c]] . log_softmax(pred2[b]) )

Fast path ("fast2") exploits the block structure of the harness table
(table[c, c*10:(c+1)*10] = u, zeros elsewhere, same u for every row):

    loss_b = BLOCK*u * lse_b - u * S[b, c*_b]
    lse_b  = log(sum_j exp(pred2[b,j]))
    S[b,c] = sum of the 10-wide block c of pred2 row b

Per 128x1000 segment the device computes only two row scalars (se = sum_j
exp(pred2) for the host-side lse, and the masked dot term), spread over
three engines so each stays under the ~87us DMA stream (36.9MB @ ~425GB/s):

 - ACT: 64 Exp ops.  On PE tiles sub-rows 0-2 merge into one [P,3000]
   instr (row-sums via a single 3-segment DVE reduce); elsewhere each
   [P,1000] exp row-sums on the ACT accumulator (read costs ~280ns).
 - PE (11 tiles, interleaved 3-on-1-off so its 6.2us/tile pace never
   builds enough lag to throttle the stream through p2-buffer recycling):
   G += onehot^T @ pred2 in PSUM; one DVE fold against a host-supplied
   block mask ships trace-sum(G) mid-stream.  One-hot is materialized
   compactly (is_ge per PE tile) as f32r LDWEIGHTS input.
 - DVE (5 tiles incl. head/tail): fused dot jobs — one scalar_tensor_tensor
   computes (pred1_bcast >= rmax) * pred2 and row-reduces into the
   accumulator, so the one-hot never materializes for these tiles.

DMA: one sync ring, priority order (tile0 k-split + pred1 quarter 0
first; pred1 quarters drop in between early tiles; block mask at the
fold's deadline; tail tiles split finer so the last segments land
continuously).  First data ~8us (hard runtime floor), stream sustains
~420-430 GB/s, outputs ship over the ACT HWDGE ring.

Notes: per-core exec is bimodal (~109us vs ~123us) due to HW utilization
throttling (see NTFF summary throttle fields) — measure min-of-N.

Sharding: data-parallel over B across 8 NeuronCores; host combines the
per-core partial outputs in f64.  Non-block tables fall back to the
previous matmul-based programs ("fast"/"general"), which handle any table.
"""

import sys
from contextlib import ExitStack

import numpy as np

for _p in ("/opt/trn_rl_repo", "/root/.axon_site/_ro/trn_rl_repo"):
    if _p not in sys.path:
        sys.path.append(_p)

import concourse.bass as bass
import concourse.tile as tile
from concourse import bacc, mybir
from concourse.bass_utils import run_bass_kernel_spmd

B, C1, C2 = 65536, 100, 1000
BLOCK = C2 // C1            # 10 fine classes per coarse class
NCORES = 8
BC = B // NCORES            # rows per core (8192)
P = 128                     # partitions
KS = 4                      # sub-rows per partition per tile
NT = BC // (P * KS)         # tiles per core (16)
NSEG = BC // P              # per-row segments per core (64) == NT*KS
NQ = 4                      # one-hot quarters (DVE op granularity)
JQ = NSEG // NQ             # segments per quarter (16)
F32 = mybir.dt.float32
F32R = mybir.dt.float32r
X = mybir.AxisListType.X
ALU = mybir.AluOpType
ACTF = mybir.ActivationFunctionType

# ---- fast2 static knobs ----------------------------------------------------
# early tiles whose dot term accumulates on the (otherwise idle) PE as
# G += onehot^T @ pred2, folded once against the static block mask; late
# tiles use the fused DVE dot job (PE would fall behind the stream there).
# Must be a prefix range (dot-column layout assumes it).
# interleaved 3-on-1-off so the PE (6.2us/tile vs 4.8us arrivals) never
# builds enough lag to throttle the DMA stream via p2-buffer recycling
PE_D_TILES = frozenset({1, 2, 3, 5, 6, 7, 9, 10, 11, 13, 14})
# compact one-hot: one slot per PE-tile segment
_PE_TILES_ORD = sorted(PE_D_TILES)
_PE_SLOT = {i: 4 * ix for ix, i in enumerate(_PE_TILES_ORD)}
N_PE_SEG = KS * len(PE_D_TILES)
# dot-term columns: one per DVE-dot segment, packed in segment order
_DVE_D_SEGS = [s for s in range(NSEG) if s // KS not in PE_D_TILES]
_D_COL = {s: NSEG + ix for ix, s in enumerate(_DVE_D_SEGS)}
ND = len(_DVE_D_SEGS)
PSUM_CHUNKS = [(0, 512), (512, C2)]


def _build_fast2() -> bass.Bass:
    nc = bacc.Bacc("TRN2", target_bir_lowering=False, debug=False,
                   num_devices=NCORES)
    p1 = nc.dram_tensor("p1", [BC, C1], F32, kind="ExternalInput").ap()
    p2 = nc.dram_tensor("p2", [BC, C2], F32, kind="ExternalInput").ap()
    sd_out = nc.dram_tensor("sd", [P, NSEG + ND], F32,
                            kind="ExternalOutput").ap()
    gd_out = nc.dram_tensor("gd", [C1, 1], F32, kind="ExternalOutput").ap()

    with tile.TileContext(nc) as tc:
        with ExitStack() as ctx:
            _fast2_body(ctx, tc, p1, p2, sd_out, gd_out)
    nc.compile()
    return nc


def _fast2_body(ctx: ExitStack, tc, p1, p2, sd_out, gd_out):
    nc = tc.nc
    consts = ctx.enter_context(tc.tile_pool(name="consts", bufs=1))
    p2pool = ctx.enter_context(tc.tile_pool(name="p2", bufs=6))
    etp3 = ctx.enter_context(tc.tile_pool(name="exp3p", bufs=4))
    etp1 = ctx.enter_context(tc.tile_pool(name="exp1p", bufs=1))
    psum = ctx.enter_context(tc.tile_pool(name="psum", bufs=1, space="PSUM"))

    # row (p*64 + i*4 + k)  <->  tile i, partition p, sub-row k
    p2t = p2.rearrange("(p i k) c -> i p (k c)", p=P, i=NT, k=KS)
    # pred1 quarters: per-partition 6.4KB contiguous runs, arriving just
    # ahead of the segments whose row-max they feed
    p1q = p1.rearrange("(p q j) c -> q p (j c)", p=P, q=NQ, j=JQ)

    p1big = consts.tile([P, NSEG * C1], F32)
    oh_pe = consts.tile([P, N_PE_SEG * C1], F32R)   # one-hot for PE tiles
    rmax = consts.tile([P, NSEG], F32)
    sd_all = consts.tile([P, NSEG + ND], F32)
    scr = consts.tile([P, C2], F32)
    bm_sb = consts.tile([C1, C2], F32)
    gscr = consts.tile([C1, C2], F32)
    gdot = consts.tile([C1, 1], F32)
    G = psum.tile([C1, C2], F32)

    p1big3 = p1big[:].rearrange("p (j c) -> p j c", j=NSEG)
    oh3 = oh_pe[:].rearrange("p (j c) -> p j c", j=N_PE_SEG)
    scr3 = scr[:].rearrange("p (c b) -> p c b", b=BLOCK)

    # block-diagonal fold mask built on the (otherwise idle) Pool engine:
    # keep 1.0 where 0 <= j - BLOCK*c <= BLOCK-1, else 0.
    nc.gpsimd.memset(gscr[:], 1.0)
    nc.gpsimd.affine_select(bm_sb[:], gscr[:], pattern=[[1, C2]],
                            compare_op=ALU.is_ge, fill=0.0, base=0,
                            channel_multiplier=-BLOCK)
    nc.gpsimd.affine_select(gscr[:], bm_sb[:], pattern=[[-1, C2]],
                            compare_op=ALU.is_ge, fill=0.0, base=BLOCK - 1,
                            channel_multiplier=BLOCK)

    def rmax_quarter(q):
        js = slice(q * JQ, (q + 1) * JQ)
        nc.vector.reduce_max(rmax[:, js], p1big3[:, js, :], axis=X)

    t2_tiles = []

    def load_tile(i, eng):
        t2 = p2pool.tile([P, KS * C2], F32R, tag="p2")
        if i == 0 or i == NT - 1:
            # k-split: pipeline head starts early / tail lands continuously
            for k in range(KS):
                eng.dma_start(t2[:, bass.ts(k, C2)],
                              p2t[i][:, bass.ts(k, C2)].bitcast(F32R))
        elif i >= NT - 3:
            # pair-split taper for the tiles feeding the tail
            for k in range(0, KS, 2):
                eng.dma_start(t2[:, k * C2:(k + 2) * C2],
                              p2t[i][:, k * C2:(k + 2) * C2].bitcast(F32R))
        else:
            eng.dma_start(t2[:], p2t[i].bitcast(F32R))
        t2_tiles.append(t2)

    def consume_tile(i):
        t2 = t2_tiles[i]
        if i in PE_D_TILES:
            # one-hot for this tile's 4 segments (f32r bits for LDWEIGHTS)
            sl = _PE_SLOT[i]
            js = slice(i * KS, (i + 1) * KS)
            rmx3 = rmax[:, js].unsqueeze(2).broadcast_to((P, KS, C1))
            nc.vector.tensor_tensor(oh3[:, sl:sl + KS, :], p1big3[:, js, :],
                                    rmx3, op=ALU.is_ge)
        for k in range(KS):
            seg = i * KS + k
            t2f = t2[:, bass.ts(k, C2)].bitcast(F32)
            se_col = sd_all[:, seg:seg + 1]
            if i not in PE_D_TILES:
                # fused dot job on the DVE: (pred1_bcast >= rmax) * pred2,
                # row-reduced into the accumulator — the one-hot never
                # materializes; 1 instr / segment.
                dot_col = sd_all[:, _D_COL[seg]:_D_COL[seg] + 1]
                p1b = p1big3[:, seg, :].unsqueeze(2).broadcast_to(
                    (P, C1, BLOCK))
                t2s = t2f.rearrange("p (c b) -> p c b", b=BLOCK)
                nc.vector.scalar_tensor_tensor(
                    scr3, p1b, rmax[:, seg:seg + 1], t2s,
                    op0=ALU.is_ge, op1=ALU.mult, accum_out=dot_col)
            if k == 0 and i in PE_D_TILES:
                # merged exp over sub-rows 0-2 (one big ACT instr, no
                # accumulator) + a single 3-segment DVE row-reduce; only
                # sub-row 3 rides the ACT accumulator.  Cuts ACT per-tile
                # cost from ~5.6us to ~4.1us on the PE tiles.
                et3 = etp3.tile([P, 3 * C2], F32, tag="exp3")
                nc.scalar.activation(et3[:], t2[:, 0:3 * C2].bitcast(F32),
                                     ACTF.Exp)
                nc.vector.reduce_sum(
                    sd_all[:, seg:seg + 3],
                    et3[:].rearrange("p (s c) -> p s c", s=3), axis=X)
            elif k in (1, 2) and i in PE_D_TILES:
                pass  # covered by the merged exp above
            else:
                et = etp1.tile([P, C2], F32, tag="exp1")
                nc.scalar.activation(et[:], t2f, ACTF.Exp, accum_out=se_col)
        if i in PE_D_TILES:
            # dot term via the PE: G += onehot^T @ pred2 (PSUM accumulate)
            for k in range(KS):
                seg = i * KS + k
                for lo, hi in PSUM_CHUNKS:
                    nc.tensor.matmul(
                        G[:, lo:hi], oh_pe[:, bass.ts(_PE_SLOT[i] + k, C1)],
                        t2[:, k * C2 + lo:k * C2 + hi],
                        start=(k == 0 and i == min(PE_D_TILES)),
                        stop=(k == KS - 1 and i == max(PE_D_TILES)))

    # --- DMA schedule (single sync ring, priority order) ---
    t0 = p2pool.tile([P, KS * C2], F32R, tag="p2")
    t2_tiles.append(t0)
    nc.sync.dma_start(t0[:, 0:C2], p2t[0][:, 0:C2].bitcast(F32R))
    nc.sync.dma_start(p1big[:, 0:JQ * C1], p1q[0])
    for k in range(1, KS):
        nc.sync.dma_start(t0[:, bass.ts(k, C2)],
                          p2t[0][:, bass.ts(k, C2)].bitcast(F32R))
    for i in range(1, NT):
        load_tile(i, nc.sync)
        if i <= 3:
            nc.sync.dma_start(p1big[:, i * JQ * C1:(i + 1) * JQ * C1], p1q[i])

    for i in range(NT):
        if i % 4 == 0:
            rmax_quarter(i // 4)
        consume_tile(i)
        if i == max(PE_D_TILES) + 1:
            # G complete after the last PE tile; fold against the block mask
            # (DVE STT reads PSUM directly) and ship mid-stream
            nc.vector.scalar_tensor_tensor(
                bm_sb[:], G[:], 1.0, gscr[:], op0=ALU.mult, op1=ALU.mult,
                accum_out=gdot[:])
            nc.scalar.dma_start(gd_out[:, :], gdot[:])

    nc.scalar.dma_start(sd_out[:, :], sd_all[:])


# ===========================================================================
# Fallback programs (arbitrary tables): previous matmul-based kernel.
# ===========================================================================

CHUNKS = [(0, 512), (512, C2)]
GA_LAST = NT - 4
ACT_ACCUM_SEGS = frozenset(s for s in range(4, 56, 3)) | {NSEG - 2, NSEG - 1}


def _build_program(general: bool) -> bass.Bass:
    nc = bacc.Bacc("TRN2", target_bir_lowering=False, debug=False,
                   num_devices=NCORES)
    p1 = nc.dram_tensor("p1", [BC, C1], F32, kind="ExternalInput").ap()
    p2 = nc.dram_tensor("p2", [BC, C2], F32, kind="ExternalInput").ap()
    tbl = nc.dram_tensor("tbl", [C1, C2], F32, kind="ExternalInput").ap()
    sbc = None
    if general:
        sbc = nc.dram_tensor("sbc", [P, C1], F32, kind="ExternalInput").ap()
    se_out = nc.dram_tensor("se", [P, NSEG], F32, kind="ExternalOutput").ap()
    rd_out = nc.dram_tensor("rd", [C1, 2], F32, kind="ExternalOutput").ap()
    sel_out = None
    if general:
        sel_out = nc.dram_tensor("sel", [P, NSEG], F32,
                                 kind="ExternalOutput").ap()

    with tile.TileContext(nc) as tc:
        with ExitStack() as ctx:
            _kernel_body(ctx, tc, p1, p2, tbl, sbc, se_out, rd_out, sel_out,
                         general)
    nc.compile()
    return nc


def _kernel_body(ctx: ExitStack, tc, p1, p2, tbl, sbc, se_out, rd_out,
                 sel_out, general):
    nc = tc.nc
    consts = ctx.enter_context(tc.tile_pool(name="consts", bufs=1))
    p2pool = ctx.enter_context(tc.tile_pool(name="p2", bufs=6))
    expp = ctx.enter_context(tc.tile_pool(name="expp", bufs=6))
    psum = ctx.enter_context(tc.tile_pool(name="psum", bufs=1, space="PSUM"))

    p2t = p2.rearrange("(p i k) c -> i p (k c)", p=P, i=NT, k=KS)
    p1h = p1.rearrange("(p h j) c -> h p (j c)", p=P, h=2, j=NSEG // 2)

    p1big = consts.tile([P, NSEG * C1], F32)
    oh_all = consts.tile([P, NSEG * C1], F32R)
    tbl_sb = consts.tile([C1, C2], F32)
    rmax = consts.tile([P, NSEG], F32)
    se_all = consts.tile([P, NSEG], F32)
    rowdots = consts.tile([C1, 2], F32)

    if general:
        sbc_sb = consts.tile([P, C1], F32)
        ss_scratch = consts.tile([P, JQ * C1], F32)
        ss3 = ss_scratch[:].rearrange("p (j c) -> p j c", j=JQ)
        sbc3 = sbc_sb[:].unsqueeze(1).broadcast_to((P, JQ, C1))
        sel_s_all = consts.tile([P, NSEG], F32)

    G_a = psum.tile([C1, C2], F32)
    G_b = psum.tile([C1, C2], F32)

    p1big3 = p1big[:].rearrange("p (j c) -> p j c", j=NSEG)
    oh3 = oh_all[:].rearrange("p (j c) -> p j c", j=NSEG)

    def onehot_quarter(q):
        js = slice(q * JQ, (q + 1) * JQ)
        nc.vector.reduce_max(rmax[:, js], p1big3[:, js, :], axis=X)
        rmx3 = rmax[:, js].unsqueeze(2).broadcast_to((P, JQ, C1))
        nc.vector.tensor_tensor(oh3[:, js, :], p1big3[:, js, :], rmx3,
                                op=ALU.is_ge)
        if general:
            nc.vector.tensor_tensor(ss3[:], oh3[:, js, :].bitcast(F32), sbc3,
                                    op=ALU.mult)
            nc.vector.reduce_sum(sel_s_all[:, js], ss3[:], axis=X)

    t2_tiles = []

    def load_tile(i):
        t2 = p2pool.tile([P, KS * C2], F32R, tag="p2")
        if i == 0 or i == NT - 1:
            for k in range(KS):
                nc.sync.dma_start(t2[:, bass.ts(k, C2)],
                                  p2t[i][:, bass.ts(k, C2)].bitcast(F32R))
        elif i >= NT - 3:
            for k in range(0, KS, 2):
                nc.sync.dma_start(t2[:, k * C2:(k + 2) * C2],
                                  p2t[i][:, k * C2:(k + 2) * C2].bitcast(F32R))
        else:
            nc.sync.dma_start(t2[:], p2t[i].bitcast(F32R))
        t2_tiles.append(t2)

    def consume_tile(i):
        t2 = t2_tiles[i]
        if i in PE_D_TILES:
            # one-hot for this tile's 4 segments (f32r bits for LDWEIGHTS)
            sl = _PE_SLOT[i]
            js = slice(i * KS, (i + 1) * KS)
            rmx3 = rmax[:, js].unsqueeze(2).broadcast_to((P, KS, C1))
            nc.vector.tensor_tensor(oh3[:, sl:sl + KS, :], p1big3[:, js, :],
                                    rmx3, op=ALU.is_ge)
        for k in range(KS):
            seg = i * KS + k
            se_col = se_all[:, seg:seg + 1]
            et = expp.tile([P, C2], F32, tag="exp1")
            if seg in ACT_ACCUM_SEGS:
                nc.scalar.activation(et[:], t2[:, bass.ts(k, C2)].bitcast(F32),
                                     ACTF.Exp, accum_out=se_col)
            else:
                nc.scalar.activation(et[:], t2[:, bass.ts(k, C2)].bitcast(F32),
                                     ACTF.Exp)
                nc.vector.reduce_sum(se_col, et[:], axis=X)
        G = G_a if i <= GA_LAST else G_b
        for k in range(KS):
            seg = i * KS + k
            for lo, hi in CHUNKS:
                nc.tensor.matmul(G[:, lo:hi], oh_all[:, bass.ts(seg, C1)],
                                 t2[:, k * C2 + lo:k * C2 + hi],
                                 start=(k == 0 and i in (0, GA_LAST + 1)),
                                 stop=(k == KS - 1 and i in (GA_LAST, NT - 1)))

    nc.sync.dma_start(p1big[:, 0:NSEG * C1 // 2], p1h[0])
    load_tile(0)
    load_tile(1)
    nc.sync.dma_start(p1big[:, NSEG * C1 // 2:], p1h[1])
    load_tile(2)
    nc.sync.dma_start(tbl_sb[:], tbl[:, :])
    if general:
        nc.sync.dma_start(sbc_sb[:], sbc[:, :])
    for i in range(3, NT):
        load_tile(i)

    gt_scratch = consts.tile([C1, C2], F32)
    for i in range(NT):
        if i % 4 == 0:
            onehot_quarter(i // 4)
        consume_tile(i)
        if i == GA_LAST + 1:
            nc.vector.tensor_mul(gt_scratch[:], G_a[:], tbl_sb[:])
            nc.vector.tensor_reduce(rowdots[:, 0:1], gt_scratch[:], axis=X,
                                    op=ALU.add, negate=True)

    nc.vector.tensor_mul(gt_scratch[:], G_b[:], tbl_sb[:])
    nc.vector.tensor_reduce(rowdots[:, 1:2], gt_scratch[:], axis=X,
                            op=ALU.add, negate=True)
    nc.sync.dma_start(se_out[:, :], se_all[:])
    if general:
        nc.sync.dma_start(sel_out[:, :], sel_s_all[:])
    nc.sync.dma_start(rd_out[:, :], rowdots[:])


_PROGRAM_CACHE: dict = {}


def _program(mode: str = "fast2") -> bass.Bass:
    if mode not in _PROGRAM_CACHE:
        if mode == "fast2":
            _PROGRAM_CACHE[mode] = _build_fast2()
        else:
            _PROGRAM_CACHE[mode] = _build_program(mode == "general")
    return _PROGRAM_CACHE[mode]


def _row_sums(table):
    return np.asarray(table, dtype=np.float32).sum(axis=1, dtype=np.float32)


def _is_uniform_s(s):
    return bool(np.all(np.abs(s - s[0]) <= 1e-6 * max(1.0, abs(float(s[0])))))


def _block_uniform_u(table):
    """u if table is the uniform block table (t[c, c*B:(c+1)*B] = u), else
    None."""
    t = np.ascontiguousarray(table, dtype=np.float32)
    if t.shape != (C1, C2):
        return None
    u = float(t[0, 0])
    if u == 0.0:
        return None
    expect = np.zeros((C1, C2), dtype=np.float32)
    for c in range(C1):
        expect[c, c * BLOCK:(c + 1) * BLOCK] = np.float32(u)
    return u if np.array_equal(t, expect) else None


def _block_mask():
    bm = np.zeros((C1, C2), dtype=np.float32)
    for c in range(C1):
        bm[c, c * BLOCK:(c + 1) * BLOCK] = 1.0
    return bm


def _in_maps(pred1_logits, pred2_logits, table, mode: str):
    p1 = np.ascontiguousarray(pred1_logits, dtype=np.float32)
    p2 = np.ascontiguousarray(pred2_logits, dtype=np.float32)
    tbl = np.ascontiguousarray(table, dtype=np.float32)
    maps = []
    for k in range(NCORES):
        m = {
            "p1": np.ascontiguousarray(p1[k * BC:(k + 1) * BC]),
            "p2": np.ascontiguousarray(p2[k * BC:(k + 1) * BC]),
        }
        if mode != "fast2":
            m["tbl"] = tbl
        if mode == "general":
            s = _row_sums(tbl)
            m["sbc"] = np.ascontiguousarray(np.tile(s, (P, 1)))
        maps.append(m)
    return maps


def _combine_fast2(result, u):
    sd = np.asarray(result["sd"], dtype=np.float64)
    se = sd[:, :NSEG]
    dot = sd[:, NSEG:].sum()
    dot += np.asarray(result["gd"], dtype=np.float64).sum()
    return (BLOCK * u) * np.log(se).sum() - u * dot


def _combine(result, s0, general):
    lse = np.log(np.asarray(result["se"], dtype=np.float64))
    if general:
        lse = lse * np.asarray(result["sel"], dtype=np.float64)
        lse_term = lse.sum()
    else:
        lse_term = s0 * lse.sum()
    return lse_term + np.asarray(result["rd"], dtype=np.float64).sum()


def run_on_device(pred1_logits, pred2_logits, table, **spmd_kwargs):
    """Compile/run the SPMD program on cores 0-7; returns (loss, results)."""
    u = _block_uniform_u(table)
    if u is not None:
        mode = "fast2"
    else:
        s = _row_sums(table)
        mode = "fast" if _is_uniform_s(s) else "general"
    nc = _program(mode)
    res = run_bass_kernel_spmd(
        nc, _in_maps(pred1_logits, pred2_logits, table, mode),
        core_ids=list(range(NCORES)), **spmd_kwargs)
    if mode == "fast2":
        partials = [_combine_fast2(r, u) for r in res.results]
    else:
        s0 = np.float64(_row_sums(table)[0])
        partials = [_combine(r, s0, mode == "general") for r in res.results]
    loss = np.float32(np.sum(partials, dtype=np.float64) / B)
    return np.asarray(loss), res


def kernel(pred1_logits, pred2_logits, table):
    loss, _ = run_on_device(pred1_logits, pred2_logits, table)
    return loss
